# revision 1
# baseline (speedup 1.0000x reference)
"""AttnTopKPool Trainium2 kernel.

reference:
    w_mean = mean(w, axis=1)          # [B, S, S] -> [B, S]
    idx    = top_k(w_mean, 16)        # [B, 16]
    out    = x[b, :, idx[b]]          # [B, F, 16]

Strategy (8 NeuronCores, batch-parallel, 4 batches each):
  - host: transpose x to x_t[b, s, f] so the device gather is a contiguous
    row gather; slice w and x_t per core.
  - device per batch (w[b] is 16 MiB, streamed once; memory-bound):
      * loads 0-2: three 4 MiB [128, 8192] tiles (4 w-rows per partition),
        accumulated elementwise on DVE into one fp32 tile (fp32 matmul
        streams at 1/4 rate, so pre-reducing 3:1 keeps TensorE off the
        critical path).
      * load 3: four 1 MiB [128, 2048] tiles fed straight to TensorE so
        the end-of-batch dependency chain is shallow.
      * column sums via TensorE ones-matmul into two PSUM groups
        (accumulator group + direct group), summed on DVE into SBUF.
        top-k of sum == top-k of mean, so no divide.
      * top-16 via DVE max8 / max_index / match_replace (two passes).
      * gather: per index, reg_load it into an SP register and issue a
        dynamic-offset HWDGE DMA fetching that 4 KiB row of x_t[b].
        (offset-table indirect DMA is not supported on this runtime.)
  - out per core: [64, 1024] = (b_loc*16 + k, f); host reassembles to
    [B, F, K].
"""

import numpy as np

B, F, S, K = 32, 1024, 2048, 16
N_CORES = 8
B_LOC = B // N_CORES  # 4
P = 128
ROWS_PER_PART = 4          # w rows per SBUF partition in one big load
LOAD_FREE = ROWS_PER_PART * S   # 8192 floats = 32 KiB per partition
N_BIG = 3                  # 4 MiB loads per batch (DVE-accumulated)
N_SMALL = 4                # small [128, 2048] loads per batch (direct MM)
MM_N = 512                 # fp32 moving-operand max / one PSUM bank
NQ = S // MM_N             # 4 psum column slices
NEG = -3.0e38              # below any column sum

_cached_nc = None

# test-only knobs (harness leaves these at defaults)
TRACE = False
_last_results = None


def _build_nc():
    from concourse import bacc, bass, mybir, tile

    f32 = mybir.dt.float32
    u32 = mybir.dt.uint32

    nc = bacc.Bacc("TRN2", target_bir_lowering=False, debug=False)

    w_d = nc.dram_tensor("w", [B_LOC, S, S], f32, kind="ExternalInput")
    xt_d = nc.dram_tensor("xt", [B_LOC, S, F], f32, kind="ExternalInput")
    out_d = nc.dram_tensor("out", [B_LOC * K, F], f32, kind="ExternalOutput")

    w_rows = w_d[:].rearrange("b r s -> (b r) s")
    # big view: [16, 128, 8192]; partition p of slot t holds rows (512t + 4p ..+3)
    w_big = w_rows.rearrange("(t p fr) s -> t p (fr s)", p=P, fr=ROWS_PER_PART)
    # small view: [64, 128, 2048]; partition p of slot m holds row (128m + p)
    w_small = w_rows.rearrange("(m p) s -> m p s", p=P)
    BIG_SLOTS_PER_B = S // (P * ROWS_PER_PART)  # 4 slots per batch; 3 used big

    with tile.TileContext(nc) as tc:
        with (
            tc.tile_pool(name="wpool", bufs=3) as wpool,
            tc.tile_pool(name="apool", bufs=1) as apool,
            tc.tile_pool(name="spool", bufs=4) as spool,
            tc.tile_pool(name="smpool", bufs=2) as smpool,
            tc.tile_pool(name="pspool", bufs=2, space="PSUM") as pspool,
            tc.tile_pool(name="tk", bufs=1) as tk,
        ):
            ones = tk.tile([P, 1], f32)
            nc.vector.memset(ones[:], 1.0)
            gath = tk.tile([B_LOC * K, F], f32)

            for b in range(B_LOC):
                # --- stream w[b]: 3 x 4 MiB loads, DVE-accumulated ---
                acc = apool.tile([P, LOAD_FREE], f32, name=f"acc{b}", tag="acc")
                prev = None
                for l in range(N_BIG):
                    wt = wpool.tile([P, LOAD_FREE], f32, name=f"wt{b}_{l}", tag="wt")
                    nc.sync.dma_start(wt[:], w_big[b * BIG_SLOTS_PER_B + l])
                    if l == 1:
                        nc.vector.tensor_add(acc[:], prev[:], wt[:])
                    elif l > 1:
                        nc.vector.tensor_add(acc[:], acc[:], wt[:])
                    prev = wt

                # --- 4 small loads fed straight to PE ---
                sts = []
                for m in range(N_SMALL):
                    st = spool.tile([P, S], f32, name=f"st{b}_{m}", tag="st")
                    nc.sync.dma_start(st[:], w_small[b * 16 + 12 + m])
                    sts.append(st)

                ps = [
                    pspool.tile([1, MM_N], f32, name=f"ps{b}_{q}", tag=f"ps{q}")
                    for q in range(NQ)
                ]
                # single accumulation group per psum slice; WAW deps on the
                # psum AP keep the start=True matmul first
                for c in range(LOAD_FREE // MM_N):
                    q = c % NQ
                    nc.tensor.matmul(
                        ps[q][:],
                        ones[:],
                        acc[:, c * MM_N : (c + 1) * MM_N],
                        start=(c < NQ),
                        stop=False,
                    )
                for m, st in enumerate(sts):
                    for q in range(NQ):
                        nc.tensor.matmul(
                            ps[q][:],
                            ones[:],
                            st[:, q * MM_N : (q + 1) * MM_N],
                            start=False,
                            stop=(m == N_SMALL - 1),
                        )

                # PSUM -> column sums in SBUF
                sums = smpool.tile([1, S], f32, name=f"sums{b}", tag="sums")
                for q in range(NQ):
                    nc.scalar.activation(
                        sums[:, q * MM_N : (q + 1) * MM_N],
                        ps[q][:],
                        mybir.ActivationFunctionType.Copy,
                    )

                # --- top-16 ---
                gidx = tk.tile([1, K], u32, name=f"gidx{b}")
                m8a = tk.tile([1, 8], f32, name=f"m8a{b}")
                nc.vector.max(m8a[:], sums[:])
                nc.vector.max_index(gidx[:, 0:8], m8a[:], sums[:])
                nc.vector.match_replace(sums[:], m8a[:], sums[:], NEG)
                m8b = tk.tile([1, 8], f32, name=f"m8b{b}")
                nc.vector.max(m8b[:], sums[:])
                nc.vector.max_index(gidx[:, 8:16], m8b[:], sums[:])

                # --- gather rows of x_t[b] via register-offset DMAs ---
                # On scalar (HWDGE, ~0.6us/op); keeps sync free so the w-load
                # queue never stalls behind a top-k-dependent reg_load. For
                # the final batch the w stream is over, so its gathers split
                # onto the idle sync queue too to halve the tail.
                for k in range(K):
                    if b == B_LOC - 1 and k % 2 == 1:
                        etype, eng = mybir.EngineType.SP, nc.sync
                    else:
                        etype, eng = mybir.EngineType.Activation, nc.scalar
                    regs = nc.alloc_registers(name=f"ri{b}_{k}", engines=(etype,))
                    reg = list(regs)[0]
                    eng.reg_load(reg, gidx[0:1, k : k + 1])
                    val = eng.snap(reg, donate=True, min_val=0, max_val=S - 1)
                    eng.dma_start(
                        gath[b * K + k : b * K + k + 1, :],
                        xt_d[b][bass.ds(val, 1), :],
                    )

                # ship this batch's 64 KiB out now; the kernel tail only
                # waits for the final batch's slice
                nc.scalar.dma_start(
                    out_d[:].rearrange("(b k) f -> b k f", k=K)[b],
                    gath[b * K : (b + 1) * K, :],
                )

    nc.compile()
    return nc


def _get_nc():
    global _cached_nc
    if _cached_nc is None:
        _cached_nc = _build_nc()
    return _cached_nc


def kernel(x: np.ndarray, w: np.ndarray) -> np.ndarray:
    from concourse import bass_utils

    x = np.asarray(x, dtype=np.float32)
    w = np.asarray(w, dtype=np.float32)
    x_t = np.ascontiguousarray(x.transpose(0, 2, 1))  # [B, S, F]

    nc = _get_nc()
    in_maps = [
        {
            "w": np.ascontiguousarray(w[c * B_LOC : (c + 1) * B_LOC]),
            "xt": x_t[c * B_LOC : (c + 1) * B_LOC],
        }
        for c in range(N_CORES)
    ]
    res = bass_utils.run_bass_kernel_spmd(
        nc, in_maps, list(range(N_CORES)), trace=TRACE
    )
    global _last_results
    _last_results = res
    out = np.concatenate([res.results[c]["out"] for c in range(N_CORES)], axis=0)
    # [B*K, F] -> [B, K, F] -> [B, F, K]
    return np.ascontiguousarray(out.reshape(B, K, F).transpose(0, 2, 1))

